# revision 1
# baseline (speedup 1.0000x reference)
"""Trainium2 Bass kernel for nn_CliquesOutputLayer (self-contained).

kernel(**inputs) -> np.ndarray [4, 160000] float32.

Sharding: one formula per NeuronCore (F = 8 = n_cores). Host-side prep packs
the atoms table as E2[n] = [batch0|batch1|batch2|batch3] x 64 f32 (1KB rows)
so one gathered row serves all 4 batches, and pre-permutes grounding indices
into the block layout the kernel consumes. Per core, chunks of 512
groundings run: 12 indirect-DMA gathers (128 rows x 1KB, f32->f32r cast in
flight), 24 PE transposes ([128,128], batch-paired), PSUM->SBUF copies, the
192->16 sigmoid layer as two accumulating f32r matmuls per batch, a
block-diagonal 16->1 second layer, and a sigmoid + store of [4,512] outputs.
"""
from contextlib import ExitStack
from dataclasses import dataclass
import numpy as np
import concourse.bass as bass
import concourse.mybir as mybir

F32 = mybir.dt.float32
F32R = mybir.dt.float32r
I32 = mybir.dt.int32


@dataclass
class Cfg:
    n_atoms: int = 100000
    g: int = 20000
    b: int = 4
    d: int = 64
    h: int = 16
    gat_bufs: int = 6
    use_f32r_mm: bool = True

    @property
    def nblk(self):
        n = (self.g + 127) // 128
        return (n + 3) // 4 * 4

    @property
    def nchunk(self):
        return self.nblk // 4

    @property
    def gpad(self):
        return self.nblk * 128


def build_nc(cfg: Cfg) -> bass.Bass:
    B, D, H = cfg.b, cfg.d, cfg.h
    BD = B * D
    NB, NC = cfg.nblk, cfg.nchunk
    BH = B * H
    mmdt = F32R if cfg.use_f32r_mm else F32
    NG = cfg.gat_bufs

    nc = bass.Bass(trn_type="TRN2")
    e2 = nc.declare_dram_parameter("e2", [cfg.n_atoms, BD], F32, isOutput=False)
    gidx = nc.declare_dram_parameter("gidx", [128, NB * 3], I32, isOutput=False)
    w01 = nc.declare_dram_parameter("w01", [2 * D, H], F32, isOutput=False)
    w12 = nc.declare_dram_parameter("w12", [D, H], F32, isOutput=False)
    w2b = nc.declare_dram_parameter("w2b", [128, B], F32, isOutput=False)
    b1x = nc.declare_dram_parameter("b1x", [H, 1], F32, isOutput=False)
    b2x = nc.declare_dram_parameter("b2x", [B, 1], F32, isOutput=False)
    iden = nc.declare_dram_parameter("iden", [128, 128], F32, isOutput=False)
    o2d = nc.declare_dram_parameter("o2d", [B, cfg.gpad], F32, isOutput=True)

    with ExitStack() as ctx:
        gat = ctx.enter_context(nc.sbuf_tensor("gat", [128, NG, 12, BD], mmdt))
        gidx_sb = ctx.enter_context(nc.sbuf_tensor("gidx_sb", [128, NB * 3], I32))
        w01_sb = ctx.enter_context(nc.sbuf_tensor("w01_sb", [2 * D, H], mmdt))
        w12_sb = ctx.enter_context(nc.sbuf_tensor("w12_sb", [D, H], mmdt))
        w2b_sb = ctx.enter_context(nc.sbuf_tensor("w2b_sb", [128, B], mmdt))
        b1_sb = ctx.enter_context(nc.sbuf_tensor("b1_sb", [H, 1], F32))
        b2_sb = ctx.enter_context(nc.sbuf_tensor("b2_sb", [B, 1], F32))
        id_sb = ctx.enter_context(nc.sbuf_tensor("id_sb", [128, 128], mmdt))
        x01 = ctx.enter_context(nc.sbuf_tensor("x01", [128, 3, 512], mmdt))
        x2 = ctx.enter_context(nc.sbuf_tensor("x2", [D, 3, 512], mmdt))
        h_sb = ctx.enter_context(nc.sbuf_tensor("h_sb", [128, 2, 512], mmdt))
        o_sb = ctx.enter_context(nc.sbuf_tensor("o_sb", [B, 2, 512], F32))
        p_t0 = ctx.enter_context(nc.psum_tensor("p_t0", [128, 1024], mmdt))
        p_t1 = ctx.enter_context(nc.psum_tensor("p_t1", [128, 1024], mmdt))
        p_t2 = ctx.enter_context(nc.psum_tensor("p_t2", [128, 1024], mmdt))
        p_h = ctx.enter_context(nc.psum_tensor("p_h", [H, 1, 512], F32))
        p_o = ctx.enter_context(nc.psum_tensor("p_o", [B, 1, 512], F32))

        io = ctx.enter_context(nc.semaphore("io"))
        wio = ctx.enter_context(nc.semaphore("wio"))
        ms_s = ctx.enter_context(nc.semaphore("ms_s"))
        gs = [ctx.enter_context(nc.semaphore(f"gs{c}")) for c in range(12)]
        st_p = [ctx.enter_context(nc.semaphore(f"st_p{i}")) for i in range(2)]
        t1_sem = ctx.enter_context(nc.semaphore("t1_sem"))
        t1b_sem = ctx.enter_context(nc.semaphore("t1b_sem"))
        t2_sem = ctx.enter_context(nc.semaphore("t2_sem"))
        cp1 = ctx.enter_context(nc.semaphore("cp1"))
        cp2 = ctx.enter_context(nc.semaphore("cp2"))
        mm_s = ctx.enter_context(nc.semaphore("mm_s"))
        mm2_s = ctx.enter_context(nc.semaphore("mm2_s"))
        hs_s = ctx.enter_context(nc.semaphore("hs_s"))
        os_s = ctx.enter_context(nc.semaphore("os_s"))
        block = ctx.enter_context(nc.Block())

        @block.sync
        def _(sync):
            sync.dma_start(out=gidx_sb[:], in_=gidx[:]).then_inc(io, 16)
            sync.dma_start(out=b1_sb[:], in_=b1x[:]).then_inc(io, 16)
            sync.dma_start(out=b2_sb[:], in_=b2x[:]).then_inc(io, 16)
            for q in range(NC):
                sync.wait_ge(os_s, q + 1)
                if q >= 2:
                    sync.wait_ge(st_p[q % 2], 16 * (q // 2))
                sync.dma_start(
                    out=o2d[:, q * 512:(q + 1) * 512], in_=o_sb[:, q % 2, :]
                ).then_inc(st_p[q % 2], 16)
            for par in range(2):
                sync.wait_ge(st_p[par], 16 * ((NC + 1 - par) // 2))

        @block.gpsimd
        def _(gpsimd):
            gpsimd.dma_start(out=w01_sb[:], in_=w01[:]).then_inc(wio, 16)
            gpsimd.dma_start(out=w12_sb[:], in_=w12[:]).then_inc(wio, 16)
            gpsimd.dma_start(out=w2b_sb[:], in_=w2b[:]).then_inc(wio, 16)
            gpsimd.dma_start(out=id_sb[:], in_=iden[:]).then_inc(wio, 16)
            gpsimd.memset(h_sb[:].bitcast(F32), 0.0).then_inc(ms_s, 1)
            gpsimd.wait_ge(io, 16 * 3)
            gpsimd.wait_ge(wio, 16 * 4)
            gpsimd.wait_ge(ms_s, 1)
            for q in range(NC):
                if q >= NG:
                    gpsimd.wait_ge(t2_sem, (q - NG + 1) * 8)
                for c in range(12):
                    i = q * 12 + c
                    if q >= 1:
                        gpsimd.wait_ge(gs[c], 16 * q)
                    gpsimd.indirect_dma_start(
                        out=gat[:, q % NG, c, :],
                        out_offset=None,
                        in_=e2[:],
                        in_offset=bass.IndirectOffsetOnAxis(
                            ap=gidx_sb[:, i:i + 1], axis=0
                        ),
                    ).then_inc(gs[c], 16)
            for c in range(12):
                gpsimd.wait_ge(gs[c], 16 * NC)

        @block.tensor
        def _(tensor):
            tensor.wait_ge(io, 16 * 3)
            tensor.wait_ge(wio, 16 * 4)
            tensor.wait_ge(ms_s, 1)
            p_t = [p_t0, p_t1, p_t2]
            t_sems = [t1_sem, t1b_sem, t2_sem]
            for q in range(NC):
                # 24 batch-paired [128,128] transposes; p_t* single-buffered:
                # wait for ALL of chunk q-1's copies before reusing the banks
                if q >= 1:
                    tensor.wait_ge(cp1, 8 * q)
                    tensor.wait_ge(cp2, 4 * q)
                for j in range(4):
                    for sl in range(3):
                        tensor.wait_ge(gs[3 * j + sl], 16 * (q + 1))
                        for hf in range(2):
                            tensor.matmul(
                                p_t[sl][:, hf * 512 + j * 128:hf * 512 + (j + 1) * 128],
                                gat[:, q % NG, 3 * j + sl, hf * 128:(hf + 1) * 128],
                                id_sb[:], is_transpose=True,
                            ).then_inc(t_sems[sl], 1)
                for bp in range(B):
                    k = q * B + bp
                    if k >= 1:
                        tensor.wait_ge(hs_s, k)       # p_h single buf
                    tensor.wait_ge(cp1, 2 * (k + 1))
                    tensor.matmul(
                        p_h[:, 0, :], w01_sb[:], x01[:, k % 3, :],
                        start=True, stop=False,
                    )
                    tensor.wait_ge(cp2, k + 1)
                    tensor.matmul(
                        p_h[:, 0, :], w12_sb[:], x2[:, k % 3, :],
                        start=False, stop=True,
                    ).then_inc(mm_s, 1)
                if q >= 1:
                    tensor.wait_ge(os_s, q)
                tensor.wait_ge(hs_s, B * (q + 1))
                tensor.matmul(
                    p_o[:, 0, :], w2b_sb[:], h_sb[:, q % 2, :],
                    start=True, stop=True,
                ).then_inc(mm2_s, 1)
            tensor.wait_ge(mm2_s, NC)

        @block.vector
        def _(vector):
            for q in range(NC):
                vector.wait_ge(t1_sem, 8 * (q + 1))
                vector.wait_ge(t1b_sem, 8 * (q + 1))
                for bp in range(B):
                    k = q * B + bp
                    if k >= 3:
                        vector.wait_ge(mm_s, k - 2)
                    pr, pc = (bp % 2) * D, (bp // 2) * 512
                    vector.tensor_copy(
                        out=x01[0:D, k % 3, :], in_=p_t0[pr:pr + D, pc:pc + 512]
                    ).then_inc(cp1, 1)
                    vector.tensor_copy(
                        out=x01[D:2 * D, k % 3, :], in_=p_t1[pr:pr + D, pc:pc + 512]
                    ).then_inc(cp1, 1)
            vector.wait_ge(cp1, 2 * NC * B)

        @block.scalar
        def _(scalar):
            for q in range(NC):
                scalar.wait_ge(t2_sem, 8 * (q + 1))
                for bp in range(B):
                    k = q * B + bp
                    if k >= 3:
                        scalar.wait_ge(mm_s, k - 2)
                    pr, pc = (bp % 2) * D, (bp // 2) * 512
                    scalar.copy(
                        out=x2[:, k % 3, :], in_=p_t2[pr:pr + D, pc:pc + 512]
                    ).then_inc(cp2, 1)
                    scalar.wait_ge(mm_s, k + 1)
                    if k >= 2 and bp == 0:
                        scalar.wait_ge(mm2_s, q - 1)
                    scalar.activation(
                        h_sb[bp * 32:bp * 32 + H, q % 2, :], p_h[:, 0, :],
                        mybir.ActivationFunctionType.Sigmoid, bias=b1_sb[:],
                    ).then_inc(hs_s, 1)
                scalar.wait_ge(mm2_s, q + 1)
                if q >= 2:
                    scalar.wait_ge(st_p[q % 2], 16 * (q // 2))
                scalar.activation(
                    o_sb[:, q % 2, :], p_o[:, 0, :],
                    mybir.ActivationFunctionType.Sigmoid, bias=b2_sb[:],
                ).then_inc(os_s, 1)
            scalar.wait_ge(os_s, NC)

    return nc


def host_inputs(cfg: Cfg, atoms_embeddings, grounding_indices, W1, b1, W2, b2):
    B, D, H = cfg.b, cfg.d, cfg.h
    Fn = grounding_indices.shape[0]
    e2 = np.ascontiguousarray(
        np.transpose(np.asarray(atoms_embeddings, np.float32), (1, 0, 2))
    ).reshape(cfg.n_atoms, B * D)
    iden = np.eye(128, dtype=np.float32)
    maps = []
    for f in range(Fn):
        gi = np.asarray(grounding_indices[f], np.int64)
        gpadded = np.zeros((cfg.gpad, 3), np.int32)
        gpadded[:cfg.g] = gi.astype(np.int32)
        gidx = np.ascontiguousarray(
            gpadded.reshape(cfg.nblk, 128, 3).transpose(1, 0, 2)
        ).reshape(128, cfg.nblk * 3)
        w1f = np.asarray(W1[f], np.float32)
        w2f = np.asarray(W2[f], np.float32)
        w2bm = np.zeros((128, B), np.float32)
        for bp in range(B):
            w2bm[bp * 32:bp * 32 + H, bp] = w2f[:, 0]
        maps.append({
            "e2": e2,
            "gidx": gidx,
            "w01": np.ascontiguousarray(w1f[:2 * D]),
            "w12": np.ascontiguousarray(w1f[2 * D:]),
            "w2b": w2bm,
            "b1x": np.asarray(b1[f], np.float32)[:, None],
            "b2x": np.full((B, 1), np.float32(np.asarray(b2[f]).ravel()[0])),
            "iden": iden,
        })
    return maps


def assemble(cfg: Cfg, results):
    Fn = len(results)
    out = np.zeros((cfg.b, Fn * cfg.g), np.float32)
    for f in range(Fn):
        out[:, f * cfg.g:(f + 1) * cfg.g] = results[f]["o2d"][:, :cfg.g]
    return out


_RUNTIME = {}


def _get_runtime():
    if "nc" not in _RUNTIME:
        cfg = Cfg()
        _RUNTIME["cfg"] = cfg
        _RUNTIME["nc"] = build_nc(cfg)
    return _RUNTIME["cfg"], _RUNTIME["nc"]


def kernel(atoms_embeddings, grounding_indices, W1, b1, W2, b2):
    from concourse.bass_utils import run_bass_kernel_spmd

    cfg, nc = _get_runtime()
    maps = host_inputs(cfg, atoms_embeddings, grounding_indices, W1, b1, W2, b2)
    res = run_bass_kernel_spmd(nc, maps, list(range(len(maps))))
    return assemble(cfg, [res.results[i] for i in range(len(maps))]).astype(np.float32)



# revision 17
# speedup vs baseline: 3.1996x; 3.1996x over previous
"""Trainium2 Bass kernel for nn_CliquesOutputLayer (self-contained).

kernel(**inputs) -> np.ndarray [4, 160000] float32.

Sharding: one formula per NeuronCore (F = 8 = n_cores).

Gather: InstDMAGatherAnt (`dma_gather`, mlp ucode library, non-transposed,
multi-packet). Q7 descriptor generation runs on one core-pair per
queue_num, so gathers round-robin queue_num 0..3 to use all 4 Q7
core-pairs concurrently (num_swdge_queues=4). Each gather fetches
NI=3072 rows = 2 compute chunks, amortizing the ~10us/pair fixed cost.
(The transposed-gather mode would skip the PE transposes below, but
concurrent transposed gathers corrupt each other through the shared
xbar, and serializing them leaves 4x descriptor-gen throughput on the
table -- non-transpose + PE transposes is strictly faster.)

dma_gather needs int16 indices, so the host compacts the atoms table per
10-chunk segment (<= 15360 draws -> unique rows fit a 16384-row table and
local ids fit int16 deterministically). Table rows pack [b0|b1|b2|b3] x 64
bf16 (512B) so one gathered row serves all 4 batches.

Compute per 512-grounding chunk: 24 PE transposes ([128 g, 128 feat] bf16
-> bf16 PSUM, batch-pair-packed), 3 PSUM->SBUF copies (DVE x2, ACT x1),
layer 1 as 6 accumulating bf16 matmuls (batch pairs (0,1)/(2,3) share
K=128 via block-diagonal W1), one [64,512] sigmoid, layer 2 as one K=64
block-diagonal matmul, one [4,512] sigmoid, store. PE/ACT software
pipelines are skewed so no engine round-trips stall PE.
"""
from contextlib import ExitStack
from dataclasses import dataclass
import numpy as np
import concourse.bass as bass
import concourse.mybir as mybir

F32 = mybir.dt.float32
BF16 = mybir.dt.bfloat16
I16 = mybir.dt.int16


@dataclass
class Cfg:
    n_atoms: int = 100000
    g: int = 20000
    b: int = 4
    d: int = 64
    h: int = 16
    chunks_per_gather: int = 2
    gat_bufs: int = 4
    seg_cap: int = 16384

    @property
    def gpad(self):
        blocks = (self.g + 127) // 128
        align = 4 * self.chunks_per_gather
        return (blocks + align - 1) // align * align * 128

    @property
    def nchunk(self):
        return self.gpad // 512

    @property
    def ngather(self):
        return self.nchunk // self.chunks_per_gather

    @property
    def chunks_per_seg(self):
        return max(1, self.seg_cap // 1536)

    @property
    def nseg(self):
        return (self.nchunk + self.chunks_per_seg - 1) // self.chunks_per_seg


def build_nc(cfg: Cfg) -> bass.Bass:
    from concourse import library_config

    B, D, H = cfg.b, cfg.d, cfg.h
    BD = B * D              # 256 elements per table row
    NC = cfg.nchunk
    NG = cfg.ngather
    CPG = cfg.chunks_per_gather
    NGB = cfg.gat_bufs
    NI = 1536 * CPG         # rows per gather
    IW = NI // 16           # idx free-dim words per gather (16-part wrap)

    nc = bass.Bass(trn_type="TRN2", num_swdge_queues=4)
    e2c = nc.declare_dram_parameter(
        "e2c", [cfg.nseg, cfg.seg_cap, BD], BF16, isOutput=False)
    gidx = nc.declare_dram_parameter("gidx", [128, NG * IW], I16, isOutput=False)
    w1blk = nc.declare_dram_parameter("w1blk", [128, 3, 2 * H], BF16, isOutput=False)
    w2q = nc.declare_dram_parameter("w2q", [4 * H, B], BF16, isOutput=False)
    b1q = nc.declare_dram_parameter("b1q", [4 * H, 1], F32, isOutput=False)
    b2q = nc.declare_dram_parameter("b2q", [B, 1], F32, isOutput=False)
    iden = nc.declare_dram_parameter("iden", [128, 128], BF16, isOutput=False)
    o2d = nc.declare_dram_parameter("o2d", [B, cfg.gpad], F32, isOutput=True)

    with ExitStack() as ctx:
        # gathered rows: [g-in-block 128, buf, block (CPG*12), row 256]
        gat = ctx.enter_context(
            nc.sbuf_tensor("gat", [128, NGB, CPG * 12, BD], BF16))
        gidx_sb = ctx.enter_context(nc.sbuf_tensor("gidx_sb", [128, NG * IW], I16))
        w1_sb = ctx.enter_context(nc.sbuf_tensor("w1_sb", [128, 3, 2 * H], BF16))
        w2_sb = ctx.enter_context(nc.sbuf_tensor("w2_sb", [4 * H, B], BF16))
        b1_sb = ctx.enter_context(nc.sbuf_tensor("b1_sb", [4 * H, 1], F32))
        b2_sb = ctx.enter_context(nc.sbuf_tensor("b2_sb", [B, 1], F32))
        id_sb = ctx.enter_context(nc.sbuf_tensor("id_sb", [128, 128], BF16))
        # post-transpose activations: [feat, pair01 512 | pair23 512] x2 bufs
        xsb = [
            ctx.enter_context(nc.sbuf_tensor(f"xsb{sl}", [128, 2, 1024], BF16))
            for sl in range(3)
        ]
        h_sb = ctx.enter_context(nc.sbuf_tensor("h_sb", [4 * H, 2, 512], BF16))
        o_sb = ctx.enter_context(nc.sbuf_tensor("o_sb", [B, 2, 512], F32))
        # PSUM: 3 transpose slabs (bf16, 1 bank per buf) + h/o slab (f32)
        pt = [
            ctx.enter_context(nc.psum_tensor(f"pt{sl}", [128, 2, 1024], BF16))
            for sl in range(3)
        ]
        pho = ctx.enter_context(nc.psum_tensor("pho", [4 * H + B, 2, 512], F32))

        io = ctx.enter_context(nc.semaphore("io"))
        wio = ctx.enter_context(nc.semaphore("wio"))
        gs = [ctx.enter_context(nc.semaphore(f"gs{c}")) for c in range(4)]
        tp_s = ctx.enter_context(nc.semaphore("tp_s"))
        cpv_s = ctx.enter_context(nc.semaphore("cpv_s"))
        cpa_s = ctx.enter_context(nc.semaphore("cpa_s"))
        mm1_s = ctx.enter_context(nc.semaphore("mm1_s"))
        hs_s = ctx.enter_context(nc.semaphore("hs_s"))
        mm2_s = ctx.enter_context(nc.semaphore("mm2_s"))
        os_s = ctx.enter_context(nc.semaphore("os_s"))
        st_p = [ctx.enter_context(nc.semaphore(f"st_p{i}")) for i in range(2)]
        block = ctx.enter_context(nc.Block())

        @block.sync
        def _(sync):
            sync.dma_start(out=gidx_sb[:], in_=gidx[:]).then_inc(io, 16)
            sync.dma_start(out=w1_sb[:], in_=w1blk[:]).then_inc(wio, 16)
            sync.dma_start(out=w2_sb[:], in_=w2q[:]).then_inc(wio, 16)
            sync.dma_start(out=b1_sb[:], in_=b1q[:]).then_inc(wio, 16)
            sync.dma_start(out=b2_sb[:], in_=b2q[:]).then_inc(wio, 16)
            sync.dma_start(out=id_sb[:], in_=iden[:]).then_inc(wio, 16)
            for q in range(NC):
                sync.wait_ge(os_s, q + 1)
                if q >= 2:
                    sync.wait_ge(st_p[q % 2], 16 * (q // 2))
                sync.dma_start(
                    out=o2d[:, q * 512:(q + 1) * 512], in_=o_sb[:, q % 2, :]
                ).then_inc(st_p[q % 2], 16)
            for par in range(2):
                sync.wait_ge(st_p[par], 16 * ((NC + 1 - par) // 2))

        @block.gpsimd
        def _(gpsimd):
            gpsimd.load_library(library_config.mlp)
            gpsimd.wait_ge(io, 16)
            for g in range(NG):
                if g >= NGB:
                    # gat buffer free when both chunks' transposes are done
                    gpsimd.wait_ge(tp_s, 24 * CPG * (g - NGB + 1))
                if g >= 4:
                    gpsimd.wait_ge(gs[g % 4], 16 * (g // 4))
                gpsimd.dma_gather(
                    out_ap=gat[:, g % NGB, :, :],
                    in_ap=e2c[(g * CPG) // cfg.chunks_per_seg],
                    idxs_ap=gidx_sb[:, g * IW:(g + 1) * IW],
                    num_idxs=NI,
                    num_idxs_reg=NI,
                    elem_size=BD,
                    transpose=False,
                    single_packet=False,
                    queue_num=g % 4,
                ).then_inc(gs[g % 4], 16)
            for c in range(4):
                gpsimd.wait_ge(gs[c], 16 * ((NG - c + 3) // 4))

        def pe_transposes(tensor, q):
            # 24 transposes: [128 g, 128 (bpair,d)] -> bf16 PSUM
            bq = q % 2
            g = q // CPG
            tensor.wait_ge(gs[g % 4], 16 * (g // 4 + 1))
            if q >= 2:
                tensor.wait_ge(cpv_s, 2 * (q - 1))
                tensor.wait_ge(cpa_s, q - 1)
            cbase = (q % CPG) * 12
            for j in range(4):
                for sl in range(3):
                    for hf in range(2):
                        tensor.matmul(
                            pt[sl][:, bq, hf * 512 + j * 128:hf * 512 + (j + 1) * 128],
                            gat[:, g % NGB, cbase + j * 3 + sl,
                                hf * 128:(hf + 1) * 128],
                            id_sb[:], is_transpose=True,
                        ).then_inc(tp_s, 1)

        def pe_layer1(tensor, q):
            # layer 1: 2 pairs x 3 slots, K=128 block-diagonal W1
            bq = q % 2
            tensor.wait_ge(cpv_s, 2 * (q + 1))
            tensor.wait_ge(cpa_s, q + 1)
            if q >= 2:
                tensor.wait_ge(hs_s, q - 1)
            for p in range(2):
                for sl in range(3):
                    mm = tensor.matmul(
                        pho[p * 2 * H:(p + 1) * 2 * H, bq, :],
                        w1_sb[:, sl, :],
                        xsb[sl][:, bq, p * 512:(p + 1) * 512],
                        start=(sl == 0), stop=(sl == 2),
                    )
                    if sl == 2:
                        mm.then_inc(mm1_s, 1)

        def pe_layer2(tensor, q):
            # layer 2: K=64, 4-batch block-diagonal W2
            bq = q % 2
            tensor.wait_ge(hs_s, q + 1)
            if q >= 2:
                tensor.wait_ge(os_s, q - 1)
            tensor.matmul(
                pho[4 * H:4 * H + B, bq, :], w2_sb[:], h_sb[:, bq, :],
                start=True, stop=True,
            ).then_inc(mm2_s, 1)

        @block.tensor
        def _(tensor):
            tensor.wait_ge(wio, 16 * 5)
            # software pipeline: T[q] | L1[q-1] | L2[q-2] keeps PE fed while
            # DVE/ACT drain PSUM and ACT computes sigmoids
            for q in range(NC):
                pe_transposes(tensor, q)
                if q >= 1:
                    pe_layer1(tensor, q - 1)
                if q >= 2:
                    pe_layer2(tensor, q - 2)
            pe_layer1(tensor, NC - 1)
            pe_layer2(tensor, NC - 2)
            pe_layer2(tensor, NC - 1)
            tensor.wait_ge(mm2_s, NC)

        @block.vector
        def _(vector):
            for q in range(NC):
                bq = q % 2
                vector.wait_ge(tp_s, 24 * (q + 1))
                if q >= 2:
                    vector.wait_ge(mm1_s, 2 * (q - 1))
                vector.tensor_copy(
                    out=xsb[0][:, bq, :], in_=pt[0][:, bq, :]
                ).then_inc(cpv_s, 1)
                vector.tensor_copy(
                    out=xsb[1][:, bq, :], in_=pt[1][:, bq, :]
                ).then_inc(cpv_s, 1)
            vector.wait_ge(cpv_s, 2 * NC)

        def act_copy(scalar, q):
            bq = q % 2
            scalar.wait_ge(tp_s, 24 * (q + 1))
            if q >= 2:
                scalar.wait_ge(mm1_s, 2 * (q - 1))
            scalar.copy(
                out=xsb[2][:, bq, :], in_=pt[2][:, bq, :]
            ).then_inc(cpa_s, 1)

        def act_hsig(scalar, q):
            bq = q % 2
            scalar.wait_ge(mm1_s, 2 * (q + 1))
            scalar.activation(
                h_sb[:, bq, :], pho[0:4 * H, bq, :],
                mybir.ActivationFunctionType.Sigmoid, bias=b1_sb[:],
            ).then_inc(hs_s, 1)

        def act_osig(scalar, q):
            bq = q % 2
            scalar.wait_ge(mm2_s, q + 1)
            if q >= 2:
                scalar.wait_ge(st_p[q % 2], 16 * (q // 2))
            scalar.activation(
                o_sb[:, bq, :], pho[4 * H:4 * H + B, bq, :],
                mybir.ActivationFunctionType.Sigmoid, bias=b2_sb[:],
            ).then_inc(os_s, 1)

        @block.scalar
        def _(scalar):
            # skewed to match the PE pipeline: C[q] | Hs[q-1] | Os[q-2]
            for q in range(NC):
                act_copy(scalar, q)
                if q >= 1:
                    act_hsig(scalar, q - 1)
                if q >= 2:
                    act_osig(scalar, q - 2)
            act_hsig(scalar, NC - 1)
            act_osig(scalar, NC - 2)
            act_osig(scalar, NC - 1)
            scalar.wait_ge(os_s, NC)

    # populate .instr bytes for InstISA subclasses (the library reload);
    # without this the NEFF compiler fails with "ISA wrong length"
    from concourse.library_overlay import lower_extended_insts
    lower_extended_insts(nc)
    return nc


def host_inputs(cfg: Cfg, atoms_embeddings, grounding_indices, W1, b1, W2, b2):
    import ml_dtypes

    B, D, H = cfg.b, cfg.d, cfg.h
    NC, CPS, CPG = cfg.nchunk, cfg.chunks_per_seg, cfg.chunks_per_gather
    NG = cfg.ngather
    NI, IW = 1536 * CPG, 1536 * CPG // 16
    Fn = grounding_indices.shape[0]
    e2 = np.ascontiguousarray(
        np.transpose(np.asarray(atoms_embeddings, np.float32), (1, 0, 2))
    ).reshape(cfg.n_atoms, B * D).astype(ml_dtypes.bfloat16)
    iden = np.eye(128, dtype=np.float32).astype(ml_dtypes.bfloat16)
    # wrap map: index position i -> (partition i%16 (replicated), word i//16)
    pmod = (np.arange(128) % 16)[:, None]
    words = np.arange(IW)[None, :]
    maps = []
    for f in range(Fn):
        gi = np.asarray(grounding_indices[f], np.int64)
        gpadded = np.zeros((cfg.gpad, 3), np.int32)
        gpadded[:cfg.g] = gi.astype(np.int32)
        e2cs = np.zeros((cfg.nseg, cfg.seg_cap, B * D), ml_dtypes.bfloat16)
        inv_all = np.zeros((cfg.gpad, 3), np.int16)
        for s in range(cfg.nseg):
            seg = gpadded[s * CPS * 512:(s + 1) * CPS * 512]
            uniq, inv = np.unique(seg, return_inverse=True)
            assert len(uniq) <= cfg.seg_cap
            e2cs[s, :len(uniq)] = e2[uniq]
            inv_all[s * CPS * 512:(s + 1) * CPS * 512] = (
                inv.reshape(seg.shape).astype(np.int16))
        gidx_cols = []
        for g in range(NG):
            # gather row index i = c*128 + p lands at gat[p, c, :], with
            # c = chunk_in_gather*12 + j*3 + sl and p = g-in-block
            seg_inv = inv_all[g * CPG * 512:(g + 1) * CPG * 512]  # [CPG*512, 3]
            arr = (seg_inv
                   .reshape(CPG, 4, 128, 3)       # [cq, j, p, sl]
                   .transpose(0, 1, 3, 2)         # [cq, j, sl, p]
                   .reshape(NI))
            gidx_cols.append(arr[words * 16 + pmod])              # [128, IW]
        gidxf = np.concatenate(gidx_cols, axis=1)                 # [128, NG*IW]
        w1f = np.asarray(W1[f], np.float32)      # [192, 16]
        w2f = np.asarray(W2[f], np.float32)      # [16, 1]
        w1b = np.zeros((128, 3, 2 * H), np.float32)
        for sl in range(3):
            w1b[0:D, sl, 0:H] = w1f[sl * D:(sl + 1) * D]
            w1b[D:2 * D, sl, H:2 * H] = w1f[sl * D:(sl + 1) * D]
        w2b = np.zeros((4 * H, B), np.float32)
        for bp in range(B):
            w2b[bp * H:(bp + 1) * H, bp] = w2f[:, 0]
        b1v = np.asarray(b1[f], np.float32)
        maps.append({
            "e2c": e2cs,
            "gidx": gidxf,
            "w1blk": w1b.astype(ml_dtypes.bfloat16),
            "w2q": w2b.astype(ml_dtypes.bfloat16),
            "b1q": np.tile(b1v, B)[:, None].copy(),
            "b2q": np.full((B, 1), np.float32(np.asarray(b2[f]).ravel()[0])),
            "iden": iden,
        })
    return maps


def assemble(cfg: Cfg, results):
    Fn = len(results)
    out = np.zeros((cfg.b, Fn * cfg.g), np.float32)
    for f in range(Fn):
        out[:, f * cfg.g:(f + 1) * cfg.g] = results[f]["o2d"][:, :cfg.g]
    return out


_RUNTIME = {}


def _get_runtime():
    if "nc" not in _RUNTIME:
        cfg = Cfg()
        _RUNTIME["cfg"] = cfg
        _RUNTIME["nc"] = build_nc(cfg)
    return _RUNTIME["cfg"], _RUNTIME["nc"]


def kernel(atoms_embeddings, grounding_indices, W1, b1, W2, b2):
    from concourse.bass_utils import run_bass_kernel_spmd

    cfg, nc = _get_runtime()
    maps = host_inputs(cfg, atoms_embeddings, grounding_indices, W1, b1, W2, b2)
    res = run_bass_kernel_spmd(nc, maps, list(range(len(maps))))
    return assemble(cfg, [res.results[i] for i in range(len(maps))]).astype(np.float32)


# revision 21
# speedup vs baseline: 3.7922x; 1.1852x over previous
"""Trainium2 Bass kernel for nn_CliquesOutputLayer (self-contained).

kernel(**inputs) -> np.ndarray [4, 160000] float32.

Sharding: one formula per NeuronCore (F = 8 = n_cores).

Gather: InstDMAGatherAnt (`dma_gather`, mlp ucode library, non-transposed,
multi-packet). Q7 descriptor generation runs on one core-pair per
queue_num, so gathers round-robin queue_num 0..3 to use all 4 Q7
core-pairs concurrently (num_swdge_queues=4). Each gather fetches
NI=3072 rows = 2 compute chunks, amortizing the ~10us/pair fixed cost.
(The transposed-gather mode would skip the PE transposes below, but
concurrent transposed gathers corrupt each other through the shared
xbar, and serializing them leaves 4x descriptor-gen throughput on the
table -- non-transpose + PE transposes is strictly faster.)

dma_gather needs int16 indices, so the host compacts the atoms table per
10-chunk segment (<= 15360 draws -> unique rows fit a 16384-row table and
local ids fit int16 deterministically). Table rows pack [b0|b1|b2|b3] x 64
bf16 (512B) so one gathered row serves all 4 batches.

Compute per 512-grounding chunk: 24 PE transposes ([128 g, 128 feat] bf16
-> bf16 PSUM, batch-pair-packed), 3 PSUM->SBUF copies (DVE x2, ACT x1),
layer 1 as 6 accumulating bf16 matmuls (batch pairs (0,1)/(2,3) share
K=128 via block-diagonal W1), one [64,512] sigmoid, layer 2 as one K=64
block-diagonal matmul, one [4,512] sigmoid, store. PE/ACT software
pipelines are skewed so no engine round-trips stall PE.
"""
from contextlib import ExitStack
from dataclasses import dataclass
import numpy as np
import concourse.bass as bass
import concourse.mybir as mybir

F32 = mybir.dt.float32
BF16 = mybir.dt.bfloat16
I16 = mybir.dt.int16


@dataclass
class Cfg:
    n_atoms: int = 100000
    g: int = 20000
    b: int = 4
    d: int = 64
    h: int = 16
    chunks_per_gather: int = 2
    gat_bufs: int = 6
    seg_cap: int = 16384

    @property
    def gpad(self):
        blocks = (self.g + 127) // 128
        align = 4 * self.chunks_per_gather
        return (blocks + align - 1) // align * align * 128

    @property
    def nchunk(self):
        return self.gpad // 512

    @property
    def ngather(self):
        return len(self.gather_plan)

    @property
    def gather_plan(self):
        # list of (start_chunk, n_chunks); big gathers first, then 1-chunk
        # gathers for a fine-grained tail. Count kept a multiple of 4 so
        # queue rounds stay balanced.
        plan = []
        nc_ = self.nchunk
        tail = min(8, nc_ % (2 * 4) + 8) if nc_ > 8 else nc_
        tail -= tail % 4
        big = nc_ - tail
        assert big % self.chunks_per_gather == 0
        c = 0
        for _ in range(big // self.chunks_per_gather):
            plan.append((c, self.chunks_per_gather))
            c += self.chunks_per_gather
        for _ in range(tail):
            plan.append((c, 1))
            c += 1
        assert c == nc_ and len(plan) % 4 == 0
        return plan

    @property
    def chunks_per_seg(self):
        return max(1, self.seg_cap // 1536)

    @property
    def nseg(self):
        return (self.nchunk + self.chunks_per_seg - 1) // self.chunks_per_seg


def build_nc(cfg: Cfg) -> bass.Bass:
    from concourse import library_config

    B, D, H = cfg.b, cfg.d, cfg.h
    BD = B * D              # 256 elements per table row
    NC = cfg.nchunk
    PLAN = cfg.gather_plan
    NG = len(PLAN)
    CPG = cfg.chunks_per_gather
    NGB = cfg.gat_bufs
    IW1 = 1536 // 16        # idx words per chunk
    # chunk -> (gather idx, offset within gather); gather -> idx-word start
    chunk_gather = {}
    g_iw0 = []
    iw = 0
    for gi, (c0, nch) in enumerate(PLAN):
        g_iw0.append(iw)
        for k in range(nch):
            chunk_gather[c0 + k] = (gi, k)
        iw += nch * IW1
    TOT_IW = iw

    nc = bass.Bass(trn_type="TRN2", num_swdge_queues=4)
    e2c = nc.declare_dram_parameter(
        "e2c", [cfg.nseg, cfg.seg_cap, BD], BF16, isOutput=False)
    gidx = nc.declare_dram_parameter("gidx", [128, TOT_IW], I16, isOutput=False)
    w1blk = nc.declare_dram_parameter("w1blk", [128, 3, 2 * H], BF16, isOutput=False)
    w2q = nc.declare_dram_parameter("w2q", [4 * H, B], BF16, isOutput=False)
    b1q = nc.declare_dram_parameter("b1q", [4 * H, 1], F32, isOutput=False)
    b2q = nc.declare_dram_parameter("b2q", [B, 1], F32, isOutput=False)
    iden = nc.declare_dram_parameter("iden", [128, 128], BF16, isOutput=False)
    o2d = nc.declare_dram_parameter("o2d", [B, cfg.gpad], F32, isOutput=True)

    with ExitStack() as ctx:
        # gathered rows: [g-in-block 128, buf, block (CPG*12), row 256]
        gat = ctx.enter_context(
            nc.sbuf_tensor("gat", [128, NGB, CPG * 12, BD], BF16))
        gidx_sb = ctx.enter_context(nc.sbuf_tensor("gidx_sb", [128, TOT_IW], I16))
        dscr = ctx.enter_context(nc.sbuf_tensor("dscr", [128, 4, BD], BF16))
        w1_sb = ctx.enter_context(nc.sbuf_tensor("w1_sb", [128, 3, 2 * H], BF16))
        w2_sb = ctx.enter_context(nc.sbuf_tensor("w2_sb", [4 * H, B], BF16))
        b1_sb = ctx.enter_context(nc.sbuf_tensor("b1_sb", [4 * H, 1], F32))
        b2_sb = ctx.enter_context(nc.sbuf_tensor("b2_sb", [B, 1], F32))
        id_sb = ctx.enter_context(nc.sbuf_tensor("id_sb", [128, 128], BF16))
        # post-transpose activations: [feat, pair01 512 | pair23 512] x2 bufs
        xsb = [
            ctx.enter_context(nc.sbuf_tensor(f"xsb{sl}", [128, 2, 1024], BF16))
            for sl in range(3)
        ]
        h_sb = ctx.enter_context(nc.sbuf_tensor("h_sb", [4 * H, 2, 512], BF16))
        o_sb = ctx.enter_context(nc.sbuf_tensor("o_sb", [B, 2, 512], F32))
        # PSUM: 3 transpose slabs (bf16, 1 bank per buf) + h/o slab (f32)
        pt = [
            ctx.enter_context(nc.psum_tensor(f"pt{sl}", [128, 2, 1024], BF16))
            for sl in range(3)
        ]
        pho = ctx.enter_context(nc.psum_tensor("pho", [4 * H + B, 2, 512], F32))

        io = ctx.enter_context(nc.semaphore("io"))
        wio = ctx.enter_context(nc.semaphore("wio"))
        gs = [ctx.enter_context(nc.semaphore(f"gs{c}")) for c in range(8)]
        tp_s = ctx.enter_context(nc.semaphore("tp_s"))
        cpv_s = ctx.enter_context(nc.semaphore("cpv_s"))
        cpa_s = ctx.enter_context(nc.semaphore("cpa_s"))
        mm1_s = ctx.enter_context(nc.semaphore("mm1_s"))
        hs_s = ctx.enter_context(nc.semaphore("hs_s"))
        mm2_s = ctx.enter_context(nc.semaphore("mm2_s"))
        os_s = ctx.enter_context(nc.semaphore("os_s"))
        st_p = [ctx.enter_context(nc.semaphore(f"st_p{i}")) for i in range(2)]
        dm = [ctx.enter_context(nc.semaphore(f"dm{i}")) for i in range(4)]
        block = ctx.enter_context(nc.Block())

        @block.sync
        def _(sync):
            sync.dma_start(out=gidx_sb[:], in_=gidx[:]).then_inc(io, 16)
            sync.dma_start(out=w1_sb[:], in_=w1blk[:]).then_inc(wio, 16)
            sync.dma_start(out=w2_sb[:], in_=w2q[:]).then_inc(wio, 16)
            sync.dma_start(out=b1_sb[:], in_=b1q[:]).then_inc(wio, 16)
            sync.dma_start(out=b2_sb[:], in_=b2q[:]).then_inc(wio, 16)
            sync.dma_start(out=id_sb[:], in_=iden[:]).then_inc(wio, 16)
            for q in range(NC):
                sync.wait_ge(os_s, q + 1)
                if q >= 2:
                    sync.wait_ge(st_p[q % 2], 16 * (q // 2))
                sync.dma_start(
                    out=o2d[:, q * 512:(q + 1) * 512], in_=o_sb[:, q % 2, :]
                ).then_inc(st_p[q % 2], 16)
            for par in range(2):
                sync.wait_ge(st_p[par], 16 * ((NC + 1 - par) // 2))

        @block.gpsimd
        def _(gpsimd):
            gpsimd.load_library(library_config.mlp)
            gpsimd.wait_ge(io, 16)
            # prime each queue's Q7 pair with a tiny dummy gather so the
            # first real round pipelines instead of serializing
            for qq in range(4):
                gpsimd.dma_gather(
                    out_ap=dscr[:, qq:qq + 1, :],
                    in_ap=e2c[0],
                    idxs_ap=gidx_sb[:, 0:8],
                    num_idxs=128, num_idxs_reg=128, elem_size=BD,
                    transpose=False, single_packet=False, queue_num=qq,
                ).then_inc(dm[qq], 16)
            for g in range(NG):
                c0, nch = PLAN[g]
                if g >= NGB:
                    # gat buffer free when its prior chunks' transposes done
                    pc0, pn = PLAN[g - NGB]
                    gpsimd.wait_ge(tp_s, 24 * (pc0 + pn))
                if g >= 8:
                    # 2 alternating sems per queue: this wait is 2 rounds
                    # back so the sequencer never stalls on it
                    gpsimd.wait_ge(gs[g % 8], 16 * (g // 8))
                gpsimd.dma_gather(
                    out_ap=gat[:, g % NGB, 0:12 * nch, :],
                    in_ap=e2c[c0 // cfg.chunks_per_seg],
                    idxs_ap=gidx_sb[:, g_iw0[g]:g_iw0[g] + nch * IW1],
                    num_idxs=1536 * nch,
                    num_idxs_reg=1536 * nch,
                    elem_size=BD,
                    transpose=False,
                    single_packet=False,
                    queue_num=g % 4,
                ).then_inc(gs[g % 8], 16)
            for c in range(8):
                gpsimd.wait_ge(gs[c], 16 * ((NG - c + 7) // 8))
            for qq in range(4):
                gpsimd.wait_ge(dm[qq], 16)

        def pe_transposes(tensor, q):
            # 24 transposes: [128 g, 128 (bpair,d)] -> bf16 PSUM
            bq = q % 2
            g, koff = chunk_gather[q]
            tensor.wait_ge(gs[g % 8], 16 * (g // 8 + 1))
            if q >= 2:
                tensor.wait_ge(cpv_s, 2 * (q - 1))
                tensor.wait_ge(cpa_s, q - 1)
            cbase = koff * 12
            for j in range(4):
                for sl in range(3):
                    for hf in range(2):
                        tensor.matmul(
                            pt[sl][:, bq, hf * 512 + j * 128:hf * 512 + (j + 1) * 128],
                            gat[:, g % NGB, cbase + j * 3 + sl,
                                hf * 128:(hf + 1) * 128],
                            id_sb[:], is_transpose=True,
                        ).then_inc(tp_s, 1)

        def pe_layer1(tensor, q):
            # layer 1: 2 pairs x 3 slots, K=128 block-diagonal W1
            bq = q % 2
            tensor.wait_ge(cpv_s, 2 * (q + 1))
            tensor.wait_ge(cpa_s, q + 1)
            if q >= 2:
                tensor.wait_ge(hs_s, q - 1)
            for p in range(2):
                for sl in range(3):
                    mm = tensor.matmul(
                        pho[p * 2 * H:(p + 1) * 2 * H, bq, :],
                        w1_sb[:, sl, :],
                        xsb[sl][:, bq, p * 512:(p + 1) * 512],
                        start=(sl == 0), stop=(sl == 2),
                    )
                    if sl == 2:
                        mm.then_inc(mm1_s, 1)

        def pe_layer2(tensor, q):
            # layer 2: K=64, 4-batch block-diagonal W2
            bq = q % 2
            tensor.wait_ge(hs_s, q + 1)
            if q >= 2:
                tensor.wait_ge(os_s, q - 1)
            tensor.matmul(
                pho[4 * H:4 * H + B, bq, :], w2_sb[:], h_sb[:, bq, :],
                start=True, stop=True,
            ).then_inc(mm2_s, 1)

        @block.tensor
        def _(tensor):
            tensor.wait_ge(wio, 16 * 5)
            # software pipeline: T[q] | L1[q-1] | L2[q-2] keeps PE fed while
            # DVE/ACT drain PSUM and ACT computes sigmoids
            for q in range(NC):
                pe_transposes(tensor, q)
                if q >= 1:
                    pe_layer1(tensor, q - 1)
                if q >= 2:
                    pe_layer2(tensor, q - 2)
            pe_layer1(tensor, NC - 1)
            pe_layer2(tensor, NC - 2)
            pe_layer2(tensor, NC - 1)
            tensor.wait_ge(mm2_s, NC)

        @block.vector
        def _(vector):
            for q in range(NC):
                bq = q % 2
                vector.wait_ge(tp_s, 24 * (q + 1))
                if q >= 2:
                    vector.wait_ge(mm1_s, 2 * (q - 1))
                vector.tensor_copy(
                    out=xsb[0][:, bq, :], in_=pt[0][:, bq, :]
                ).then_inc(cpv_s, 1)
                vector.tensor_copy(
                    out=xsb[1][:, bq, :], in_=pt[1][:, bq, :]
                ).then_inc(cpv_s, 1)
            vector.wait_ge(cpv_s, 2 * NC)

        def act_copy(scalar, q):
            bq = q % 2
            scalar.wait_ge(tp_s, 24 * (q + 1))
            if q >= 2:
                scalar.wait_ge(mm1_s, 2 * (q - 1))
            scalar.copy(
                out=xsb[2][:, bq, :], in_=pt[2][:, bq, :]
            ).then_inc(cpa_s, 1)

        def act_hsig(scalar, q):
            bq = q % 2
            scalar.wait_ge(mm1_s, 2 * (q + 1))
            scalar.activation(
                h_sb[:, bq, :], pho[0:4 * H, bq, :],
                mybir.ActivationFunctionType.Sigmoid, bias=b1_sb[:],
            ).then_inc(hs_s, 1)

        def act_osig(scalar, q):
            bq = q % 2
            scalar.wait_ge(mm2_s, q + 1)
            if q >= 2:
                scalar.wait_ge(st_p[q % 2], 16 * (q // 2))
            scalar.activation(
                o_sb[:, bq, :], pho[4 * H:4 * H + B, bq, :],
                mybir.ActivationFunctionType.Sigmoid, bias=b2_sb[:],
            ).then_inc(os_s, 1)

        @block.scalar
        def _(scalar):
            # skewed to match the PE pipeline: C[q] | Hs[q-1] | Os[q-2]
            for q in range(NC):
                act_copy(scalar, q)
                if q >= 1:
                    act_hsig(scalar, q - 1)
                if q >= 2:
                    act_osig(scalar, q - 2)
            act_hsig(scalar, NC - 1)
            act_osig(scalar, NC - 2)
            act_osig(scalar, NC - 1)
            scalar.wait_ge(os_s, NC)

    # populate .instr bytes for InstISA subclasses (the library reload);
    # without this the NEFF compiler fails with "ISA wrong length"
    from concourse.library_overlay import lower_extended_insts
    lower_extended_insts(nc)
    return nc


def host_inputs(cfg: Cfg, atoms_embeddings, grounding_indices, W1, b1, W2, b2):
    import ml_dtypes

    B, D, H = cfg.b, cfg.d, cfg.h
    NC, CPS = cfg.nchunk, cfg.chunks_per_seg
    PLAN = cfg.gather_plan
    Fn = grounding_indices.shape[0]
    e2 = np.ascontiguousarray(
        np.transpose(np.asarray(atoms_embeddings, np.float32), (1, 0, 2))
    ).reshape(cfg.n_atoms, B * D).astype(ml_dtypes.bfloat16)
    iden = np.eye(128, dtype=np.float32).astype(ml_dtypes.bfloat16)
    # wrap map: index position i -> (partition i%16 (replicated), word i//16)
    pmod = (np.arange(128) % 16)[:, None]
    maps = []
    for f in range(Fn):
        gi = np.asarray(grounding_indices[f], np.int64)
        gpadded = np.zeros((cfg.gpad, 3), np.int32)
        gpadded[:cfg.g] = gi.astype(np.int32)
        e2cs = np.zeros((cfg.nseg, cfg.seg_cap, B * D), ml_dtypes.bfloat16)
        inv_all = np.zeros((cfg.gpad, 3), np.int16)
        for s in range(cfg.nseg):
            seg = gpadded[s * CPS * 512:(s + 1) * CPS * 512]
            uniq, inv = np.unique(seg, return_inverse=True)
            assert len(uniq) <= cfg.seg_cap
            e2cs[s, :len(uniq)] = e2[uniq]
            inv_all[s * CPS * 512:(s + 1) * CPS * 512] = (
                inv.reshape(seg.shape).astype(np.int16))
        gidx_cols = []
        for c0, nch in PLAN:
            # gather row index i = c*128 + p lands at gat[p, c, :], with
            # c = chunk_in_gather*12 + j*3 + sl and p = g-in-block
            ni = 1536 * nch
            seg_inv = inv_all[c0 * 512:(c0 + nch) * 512]  # [nch*512, 3]
            arr = (seg_inv
                   .reshape(nch, 4, 128, 3)       # [cq, j, p, sl]
                   .transpose(0, 1, 3, 2)         # [cq, j, sl, p]
                   .reshape(ni))
            words = np.arange(ni // 16)[None, :]
            gidx_cols.append(arr[words * 16 + pmod])              # [128, ni/16]
        gidxf = np.concatenate(gidx_cols, axis=1)
        w1f = np.asarray(W1[f], np.float32)      # [192, 16]
        w2f = np.asarray(W2[f], np.float32)      # [16, 1]
        w1b = np.zeros((128, 3, 2 * H), np.float32)
        for sl in range(3):
            w1b[0:D, sl, 0:H] = w1f[sl * D:(sl + 1) * D]
            w1b[D:2 * D, sl, H:2 * H] = w1f[sl * D:(sl + 1) * D]
        w2b = np.zeros((4 * H, B), np.float32)
        for bp in range(B):
            w2b[bp * H:(bp + 1) * H, bp] = w2f[:, 0]
        b1v = np.asarray(b1[f], np.float32)
        maps.append({
            "e2c": e2cs,
            "gidx": gidxf,
            "w1blk": w1b.astype(ml_dtypes.bfloat16),
            "w2q": w2b.astype(ml_dtypes.bfloat16),
            "b1q": np.tile(b1v, B)[:, None].copy(),
            "b2q": np.full((B, 1), np.float32(np.asarray(b2[f]).ravel()[0])),
            "iden": iden,
        })
    return maps


def assemble(cfg: Cfg, results):
    Fn = len(results)
    out = np.zeros((cfg.b, Fn * cfg.g), np.float32)
    for f in range(Fn):
        out[:, f * cfg.g:(f + 1) * cfg.g] = results[f]["o2d"][:, :cfg.g]
    return out


_RUNTIME = {}


def _get_runtime():
    if "nc" not in _RUNTIME:
        cfg = Cfg()
        _RUNTIME["cfg"] = cfg
        _RUNTIME["nc"] = build_nc(cfg)
    return _RUNTIME["cfg"], _RUNTIME["nc"]


def kernel(atoms_embeddings, grounding_indices, W1, b1, W2, b2):
    from concourse.bass_utils import run_bass_kernel_spmd

    cfg, nc = _get_runtime()
    maps = host_inputs(cfg, atoms_embeddings, grounding_indices, W1, b1, W2, b2)
    res = run_bass_kernel_spmd(nc, maps, list(range(len(maps))))
    return assemble(cfg, [res.results[i] for i in range(len(maps))]).astype(np.float32)
